# revision 2
# baseline (speedup 1.0000x reference)
"""Trainium2 Bass kernel for nn_AttentionFuser (B=16, L=1024, D=512).

Strategy: pure data-parallel over batch across 8 NeuronCores (2 batches
per core, no collectives).  Host pre-transposes activations and weights
to bf16 (the TensorEngine contracts along the partition dim, so both
operands need the contraction dim on partitions), computes the residual
sums (m1+m2 per symmetric block), and fills the pass-through output
columns (txt/au/vi) directly.  On-chip: 21 projections + 7 attentions
per pair of batches, bf16 matmuls with f32 PSUM accumulation, softmax
row-sums via ACT accum_out, probability transpose via DMA xbar.
"""

import numpy as np
import ml_dtypes

from concourse import bacc, bass, tile, mybir
from concourse.bass_utils import run_bass_kernel_spmd

B, L, D = 16, 1024, 512
A = D
NCORES = 8
BLOC = B // NCORES          # batches per core
P = 128                     # partitions
DC = D // P                 # 4 contraction chunks
AC = A // P                 # 4 attention-dim chunks
LT = L // P                 # 8 l-tiles (query tiles)
KC = L // P                 # 8 k-chunks
NH = 512                    # matmul moving-dim max (one PSUM bank of f32)
SCALE = float(1.0 / np.sqrt(np.float32(D)))

F32 = mybir.dt.float32
BF16 = mybir.dt.bfloat16
EXP = mybir.ActivationFunctionType.Exp
COPY = mybir.ActivationFunctionType.Copy

# order of the 21 stacked (pre-transposed) projection weights
W_NAMES = [f"{blk}_{w}" for blk in ("ta", "va", "tv")
           for w in ("kx", "qx", "vx", "ky", "qy", "vy")] + [
    "tav_k", "tav_q", "tav_v"]


def _build():
    nc = bacc.Bacc("TRN2", target_bir_lowering=False, debug=False,
                   num_devices=NCORES)

    mt_txt = nc.dram_tensor("mt_txt", (BLOC, D, L), BF16, kind="ExternalInput").ap()
    mt_au = nc.dram_tensor("mt_au", (BLOC, D, L), BF16, kind="ExternalInput").ap()
    mt_vi = nc.dram_tensor("mt_vi", (BLOC, D, L), BF16, kind="ExternalInput").ap()
    res = nc.dram_tensor("res", (3, BLOC, L, D), F32, kind="ExternalInput").ap()
    wt = nc.dram_tensor("wt", (21, D, A), BF16, kind="ExternalInput").ap()
    out = nc.dram_tensor("out", (BLOC, L, 4 * A), F32, kind="ExternalOutput").ap()

    with tile.TileContext(nc) as tc:
        _body(nc, tc, mt_txt, mt_au, mt_vi, res, wt, out)

    nc.compile()
    return nc


def _body(nc, tc, mt_txt, mt_au, mt_vi, res, wt, out):
    mt_dram = {"txt": mt_txt, "au": mt_au, "vi": mt_vi}

    with (
        tc.tile_pool(name="persist", bufs=1) as persist,
        tc.tile_pool(name="wpool", bufs=1) as wpool,
        tc.tile_pool(name="mpool", bufs=1) as mpool,
        tc.tile_pool(name="proj", bufs=1) as projp,
        tc.tile_pool(name="attn", bufs=2) as attnp,
        tc.tile_pool(name="small", bufs=3) as smallp,
        tc.tile_pool(name="ps_big", bufs=2, space=bass.MemorySpace.PSUM) as psb,
        tc.tile_pool(name="ps_small", bufs=4, space=bass.MemorySpace.PSUM) as pss,
    ):
        avT = [persist.tile([P, AC, L], BF16, tag=f"avT{b}", name=f"avT{b}")
               for b in range(BLOC)]

        def load_w(j):
            t = wpool.tile([P, DC, A], BF16, tag=f"w{j % 6}")
            nc.sync.dma_start(out=t[:, :, :],
                              in_=wt[j].rearrange("(dc p) a -> p dc a", p=P))
            return t

        def load_mt(name, b, slot):
            t = mpool.tile([P, DC, L], BF16, tag=f"mT{slot}_{b}")
            nc.sync.dma_start(out=t[:, :, :],
                              in_=mt_dram[name][b].rearrange("(dc p) l -> p dc l", p=P))
            return t

        def proj_T(wtile, mtile, tag):
            # out[a, l] = sum_d wT[d, a] * mT[d, l]   -> [P, AC, L] bf16
            o = projp.tile([P, AC, L], BF16, tag=tag)
            for ac in range(AC):
                for h in range(L // NH):
                    ps = pss.tile([P, NH], F32, tag="ps_s")
                    for dc in range(DC):
                        nc.tensor.matmul(ps[:, :],
                                         wtile[:, dc, ac * P:(ac + 1) * P],
                                         mtile[:, dc, h * NH:(h + 1) * NH],
                                         start=(dc == 0), stop=(dc == DC - 1))
                    nc.vector.tensor_copy(o[:, ac, h * NH:(h + 1) * NH], ps[:, :])
            return o

        def proj_N(wtile, mtile, tag):
            # out[l, a] = sum_d mT[d, l] * wT[d, a]   -> [P, KC, A] bf16
            o = projp.tile([P, KC, A], BF16, tag=tag)
            for lt in range(LT):
                ps = pss.tile([P, NH], F32, tag="ps_s")
                for dc in range(DC):
                    nc.tensor.matmul(ps[:, :],
                                     mtile[:, dc, lt * P:(lt + 1) * P],
                                     wtile[:, dc, :],
                                     start=(dc == 0), stop=(dc == DC - 1))
                nc.vector.tensor_copy(o[:, lt, :], ps[:, :])
            return o

        def attention(qT, kT, v, writer):
            # scores^T-free layout: s[q, k] tiles with q on partitions,
            # exp+rowsum on ACT, probs transposed to [k, q] via DMA xbar.
            probsT = attnp.tile([P, KC, L], BF16, tag="probsT")
            sums = smallp.tile([P, LT], F32, tag="sums")
            recip = smallp.tile([P, LT], F32, tag="recip")
            for qt in range(LT):
                ps = psb.tile([P, L], F32, tag="scores")
                for kh in range(L // NH):
                    for ac in range(AC):
                        nc.tensor.matmul(ps[:, kh * NH:(kh + 1) * NH],
                                         qT[:, ac, qt * P:(qt + 1) * P],
                                         kT[:, ac, kh * NH:(kh + 1) * NH],
                                         start=(ac == 0), stop=(ac == AC - 1))
                probs = attnp.tile([P, L], BF16, tag="probs")
                nc.scalar.activation(probs[:, :], ps[:, :], EXP, scale=SCALE,
                                     accum_out=sums[:, qt:qt + 1])
                nc.scalar.dma_start_transpose(
                    out=probsT[:, :, qt * P:(qt + 1) * P], in_=probs[:, :])
                nc.vector.reciprocal(recip[:, qt:qt + 1], sums[:, qt:qt + 1])
            for qt in range(LT):
                po = pss.tile([P, A], F32, tag="ps_s")
                for kc in range(KC):
                    nc.tensor.matmul(po[:, :],
                                     probsT[:, kc, qt * P:(qt + 1) * P],
                                     v[:, kc, :],
                                     start=(kc == 0), stop=(kc == KC - 1))
                writer(qt, po, recip[:, qt:qt + 1])

        # ---- three symmetric blocks ----
        # (block idx, m1, m2, output column); out cols: 0=ta 1=tv 2=av 3=tav
        blocks = [(0, "txt", "au", 0), (1, "vi", "au", 2), (2, "txt", "vi", 1)]
        for blk, n1, n2, col in blocks:
            w = [load_w(blk * 6 + j) for j in range(6)]  # kx qx vx ky qy vy
            for b in range(BLOC):
                m1T = load_mt(n1, b, 1)
                m2T = load_mt(n2, b, 2)
                k1T = proj_T(w[0], m1T, "k1T")
                q2T = proj_T(w[4], m2T, "q2T")
                v1 = proj_N(w[2], m1T, "v1")
                k2T = proj_T(w[3], m2T, "k2T")
                q1T = proj_T(w[1], m1T, "q1T")
                v2 = proj_N(w[5], m2T, "v2")

                o1n = projp.tile([P, LT, A], BF16, tag="o1n")

                def writer1(qt, po, rc):
                    nc.scalar.activation(o1n[:, qt, :], po[:, :], COPY, scale=rc)

                def writer2(qt, po, rc, blk=blk, b=b, col=col):
                    o2n = smallp.tile([P, A], BF16, tag="o2n")
                    nc.scalar.activation(o2n[:, :], po[:, :], COPY, scale=rc)
                    res_t = smallp.tile([P, A], F32, tag="res_t")
                    nc.sync.dma_start(
                        out=res_t[:, :],
                        in_=res[blk, b, qt * P:(qt + 1) * P, :])
                    osum = smallp.tile([P, A], F32, tag="osum")
                    nc.vector.tensor_add(osum[:, :], o1n[:, qt, :], o2n[:, :])
                    out_t = smallp.tile([P, A], F32, tag="out_t")
                    nc.vector.tensor_add(out_t[:, :], osum[:, :], res_t[:, :])
                    nc.sync.dma_start(
                        out=out[b, qt * P:(qt + 1) * P, col * A:(col + 1) * A],
                        in_=out_t[:, :])
                    if blk == 1:  # va block output is `av`, cross-attn queries
                        av_bf = smallp.tile([P, A], BF16, tag="av_bf")
                        nc.vector.tensor_copy(av_bf[:, :], out_t[:, :])
                        nc.scalar.dma_start_transpose(
                            out=avT[b][:, :, qt * P:(qt + 1) * P],
                            in_=av_bf[:, :])

                attention(q2T, k1T, v1, writer1)
                attention(q1T, k2T, v2, writer2)

        # ---- cross attention: x=txt (k, v), queries=av ----
        wk = load_w(18)
        wq = load_w(19)
        wv = load_w(20)
        for b in range(BLOC):
            xT = load_mt("txt", b, 1)
            kTc = proj_T(wk, xT, "k1T")
            qTc = proj_T(wq, avT[b], "q2T")
            vc = proj_N(wv, xT, "v1")

            def writer_c(qt, po, rc, b=b):
                out_t = smallp.tile([P, A], F32, tag="out_t")
                nc.scalar.activation(out_t[:, :], po[:, :], COPY, scale=rc)
                nc.sync.dma_start(
                    out=out[b, qt * P:(qt + 1) * P, 3 * A:4 * A],
                    in_=out_t[:, :])

            attention(qTc, kTc, vc, writer_c)


_nc_cache = None
last_results = None


def _get_nc():
    global _nc_cache
    if _nc_cache is None:
        _nc_cache = _build()
    return _nc_cache


def kernel(**inputs):
    global last_results
    txt = np.asarray(inputs["txt"], dtype=np.float32)
    au = np.asarray(inputs["au"], dtype=np.float32)
    vi = np.asarray(inputs["vi"], dtype=np.float32)

    nat = {"txt": txt, "au": au, "vi": vi}
    mt = {n: np.ascontiguousarray(v.transpose(0, 2, 1)).astype(ml_dtypes.bfloat16)
          for n, v in nat.items()}
    wt_all = np.ascontiguousarray(
        np.stack([np.asarray(inputs[n], dtype=np.float32).T for n in W_NAMES])
    ).astype(ml_dtypes.bfloat16)
    res_all = np.stack([txt + au, vi + au, txt + vi])  # (3, B, L, D) f32

    in_maps = []
    for c in range(NCORES):
        sl = slice(c * BLOC, (c + 1) * BLOC)
        in_maps.append({
            "mt_txt": mt["txt"][sl],
            "mt_au": mt["au"][sl],
            "mt_vi": mt["vi"][sl],
            "res": np.ascontiguousarray(res_all[:, sl]),
            "wt": wt_all,
        })

    nc = _get_nc()
    last_results = run_bass_kernel_spmd(nc, in_maps, core_ids=list(range(NCORES)))
    core_out = np.concatenate(
        [np.asarray(last_results.results[c]["out"]) for c in range(NCORES)], axis=0)
    return np.concatenate([txt, au, vi, core_out], axis=-1).astype(np.float32)
